# revision 16
# baseline (speedup 1.0000x reference)
"""Trainium2 Bass kernel for one decoder step (embedding + Bahdanau attention +
GRU cell + vocab projection), distributed over 8 NeuronCores.

Sharding:
  - Attention is data-parallel over batch (8 rows/core); the per-core context
    vectors are exchanged with one AllGather.
  - The GRU step is replicated over the full batch on every core (tiny).
  - emb gather happens on-device (64 row DMAs with token offsets baked into the
    program at trace time); W_fc/b_fc are sharded along the vocab axis, each
    core computing a [64, 4000] logits shard that the host concatenates.

Layout convention on device: "T" tensors are feature-major (features on SBUF
partitions, batch on the free axis). A feature index f maps to partition
f % 128, free-chunk f // 128.
"""

import numpy as np

import bass_rust
import concourse.bass as bass
import concourse.mybir as mybir
import concourse.tile as tile
from concourse.masks import make_identity

# ---------------------------------------------------------------------------
# Workaround for a walrus codegen limit: the TileContext tail drain receives
# one semaphore wait per logical processor, but TPB_CTRL (Drain) codegen only
# accepts a couple of sync waits.  Move the waits onto standalone
# EventSemaphore instructions emitted before a wait-free drain.
# ---------------------------------------------------------------------------


def _patched_drain_and_barrier(self, tick_clock, wait_clock):
    nc = self.nc
    alloc = list(self.sems.allocated().values())
    if alloc:
        probe = nc.sync.wait_ge(alloc[0], 0)
        wait_clock.add_sem_waits(
            probe.ins, bass_rust.ScopedClock({None: tick_clock.global_clock})
        )
        si = probe.ins.sync_info
        waits = list(si.on_wait) if si is not None else []
        probe.ins.sync_info = mybir.SyncInfo(on_wait=waits[:1], on_update=[])
        num2h = {h.num: h for h in alloc}
        for w in waits[1:]:
            nc.sync.wait_ge(num2h[w.id], w.wait_value)
    nc.sync.drain()
    nc.all_engine_barrier()
    assert self.sems is not None
    popped = nc._tile_sem_poison_stack.pop()
    assert popped is self._sem_poison
    nc.clear_and_free_semaphores(list(self.sems.allocated().values()))
    nc.all_engine_barrier()


tile.TileContext._drain_and_barrier = _patched_drain_and_barrier

def _split_excess_waits(nc):
    """walrus codegen accepts at most 1 sem-wait per instruction (2 for
    EventSemaphore), but Tile's sem assignment can attach many more.  Spill the
    excess onto EventSemaphore instructions inserted just before (same engine
    => executes first, so semantics are preserved)."""
    n = 0
    for fn in nc.m.functions:
        for bb in fn.blocks:
            insns = bb.instructions
            i = 0
            while i < len(insns):
                ins = insns[i]
                cap = 2 if isinstance(ins, mybir.InstEventSemaphore) else 1
                si = ins.sync_info
                waits = list(si.on_wait) if si is not None else []
                if len(waits) > cap:
                    keep = waits[: cap - 1] if cap > 1 else []
                    spill = waits[len(keep) :]
                    # leave one slot so the last spill EVSEM isn't needed
                    keep = keep + [spill.pop()]
                    ins.sync_info = mybir.SyncInfo(
                        on_wait=keep, on_update=list(si.on_update)
                    )
                    while spill:
                        chunk, spill = spill[:2], spill[2:]
                        ev = mybir.InstEventSemaphore(
                            name=f"IW-{nc.next_id()}",
                            engine=ins.engine,
                            ins=[],
                            outs=[],
                            sync_info=mybir.SyncInfo(on_wait=chunk, on_update=[]),
                            debug=ins.debug,
                        )
                        nc.register_instruction(ev, overwrite=True)
                        insns.insert(i, ev)
                        i += 1
                        n += 1
                i += 1
    return n


# ---------------------------------------------------------------------------

F32 = mybir.dt.float32
AF = mybir.ActivationFunctionType
OP = mybir.AluOpType
AX = mybir.AxisListType

B, S, V, E, Eh, H, A = 64, 128, 32000, 256, 512, 512, 256
NC = 8
BB = B // NC          # batch rows per core (attention DP)
BV = V // NC          # vocab cols per core (FC shard)
KX = (E + Eh) // 128  # 6   k-chunks of x = [emb; ctx]
KH = H // 128         # 4   k-chunks of h
KALL = KX + KH        # 10  k-chunks of [x; h] / of [h; ctx; emb]
NF = 500              # FC matmul free-dim chunk (BV = 8 * 500)

LAST_RESULT = None    # BassKernelResults of the most recent run (for test.py)
DEBUG = False         # add intermediate-tensor outputs for debugging


def _build_program(tokens):
    """Trace the SPMD program. `tokens` (python ints, len 64) are baked into the
    embedding-gather DMA offsets."""
    nc = bass.Bass()
    d = {}

    def inp(name, shape):
        d[name] = nc.declare_dram_parameter(name, list(shape), F32, isOutput=False)
        return d[name]

    def outp(name, shape):
        d[name] = nc.declare_dram_parameter(name, list(shape), F32, isOutput=True)
        return d[name]

    # per-core inputs
    enc_d = inp("enc_own", [BB, S, Eh])          # [8, 128, 512]
    hT_own_d = inp("hT_own", [Eh, BB])           # [512, 8] (columns = own batch)
    mask_d = inp("mask_own", [BB, S])            # 0 / -1e9
    wfc_d = inp("wfc_shard", [KALL * 128, BV])   # [1280, 4000]
    bfc_d = inp("bfc_rep", [B, BV])              # bias replicated over batch
    # replicated inputs
    emb_d = inp("emb", [V, E])
    waenc_d = inp("wa_enc", [Eh, A])
    wadec_d = inp("wa_dec", [H, A])
    ba_d = inp("ba", [A])
    vatt_d = inp("v_att", [A])
    wrz_d = inp("w_rzT", [(E + Eh + H), 2 * H])  # [1280, 1024]
    wnx_d = inp("w_nxT", [E + Eh, H])            # [768, 512]
    wnh_d = inp("w_nhT", [H, H])                 # [512, 512]
    brz_d = inp("b_rz_rep", [B, 2 * H])
    bnx_d = inp("b_nx_rep", [B, H])
    bnh_d = inp("b_nh_rep", [B, H])
    hid_d = inp("hidden", [B, H])
    hidT_d = inp("hiddenT", [H, B])
    # outputs
    logits_d = outp("logits", [B, BV])
    hnew_d = outp("new_hidden", [B, H])
    attn_d = outp("attn_own", [BB, S])

    with tile.TileContext(nc) as tc:
        with (
            tc.tile_pool(name="const", bufs=1) as const,
            tc.tile_pool(name="persist", bufs=1) as persist,
            tc.tile_pool(name="work", bufs=2) as work,
            tc.tile_pool(name="wg", bufs=4) as wg,          # GRU weight stream
            tc.tile_pool(name="wfc", bufs=8) as wfcp,       # FC weight stream
            tc.tile_pool(name="dram", bufs=1, space="DRAM") as dram,
            tc.tile_pool(name="ps_fc", bufs=4, space="PSUM") as ps_fc,
            tc.tile_pool(name="ps_att", bufs=2, space="PSUM") as ps_att,
            tc.tile_pool(name="ps_misc", bufs=2, space="PSUM") as ps_misc,
        ):
            ident = const.tile([128, 128], F32)
            make_identity(nc, ident)

            # ---------------- embedding gather (all 64 tokens) -------------
            emb_rows = persist.tile([128, E], F32)          # rows 0:64 = emb[tok]
            nc.vector.memset(emb_rows[64:128, :], 0.0)
            for b in range(B):
                t = int(tokens[b])
                nc.sync.dma_start(out=emb_rows[b : b + 1, :], in_=emb_d[t : t + 1, :])
            embT = persist.tile([128, E // 128, B], F32)    # [128, 2, 64]
            for o in range(E // 128):
                pt = ps_misc.tile([128, 128], F32, tag="mt")
                nc.tensor.transpose(pt[:], emb_rows[:, o * 128 : (o + 1) * 128], ident[:])
                nc.vector.tensor_copy(out=embT[:, o, :], in_=pt[:, 0:B])

            # ---------------- attention: energy = tanh(enc@Wa_enc + hd + ba)
            encT = persist.tile([128, Eh // 128, BB * S], F32)   # [128, 4, 1024]
            enc_T_src = enc_d.rearrange("b s (o p) -> p o (b s)", p=128)
            for o in range(Eh // 128):
                nc.sync.dma_start(out=encT[:, o, :], in_=enc_T_src[:, o, :])
            enc_sbe = persist.tile([128, BB, Eh], F32)           # [s=128, b, e]
            nc.sync.dma_start(out=enc_sbe[:], in_=enc_d.rearrange("b s e -> s b e"))

            waenc = persist.tile([128, Eh // 128, A], F32)
            nc.sync.dma_start(out=waenc[:], in_=waenc_d.rearrange("(o p) a -> p o a", p=128))
            wadec = persist.tile([128, H // 128, A], F32)
            nc.sync.dma_start(out=wadec[:], in_=wadec_d.rearrange("(o p) a -> p o a", p=128))
            hTo = persist.tile([128, H // 128, BB], F32)
            nc.sync.dma_start(out=hTo[:], in_=hT_own_d.rearrange("(o p) b -> p o b", p=128))
            ba_sb = const.tile([128, A // 128], F32)
            nc.sync.dma_start(out=ba_sb[:], in_=ba_d.rearrange("(o p) -> p o", p=128))
            v_sb = const.tile([128, A // 128], F32)
            nc.sync.dma_start(out=v_sb[:], in_=vatt_d.rearrange("(o p) -> p o", p=128))

            # hdT[a, b] = (hidden_own @ Wa_dec)^T
            hd_sb = persist.tile([128, A // 128, BB], F32)
            for m in range(A // 128):
                pt = ps_misc.tile([128, BB], F32, tag="mt")
                for k in range(H // 128):
                    nc.tensor.matmul(
                        pt[:],
                        wadec[:, k, m * 128 : (m + 1) * 128],
                        hTo[:, k, :],
                        start=(k == 0),
                        stop=(k == H // 128 - 1),
                    )
                nc.vector.tensor_copy(out=hd_sb[:, m, :], in_=pt[:])

            # energyT [a=256, (b s)=1024] in two a-chunks
            energy = persist.tile([128, A // 128, BB * S], F32)
            for m in range(A // 128):
                for ne in range(2):
                    pe = ps_att.tile([128, 512], F32, tag="att")
                    for k in range(Eh // 128):
                        nc.tensor.matmul(
                            pe[:],
                            waenc[:, k, m * 128 : (m + 1) * 128],
                            encT[:, k, ne * 512 : (ne + 1) * 512],
                            start=(k == 0),
                            stop=(k == Eh // 128 - 1),
                        )
                    # add hd (broadcast over s), psum -> sbuf
                    nc.vector.tensor_tensor(
                        out=energy[:, m, ne * 512 : (ne + 1) * 512].rearrange(
                            "p (b s) -> p b s", s=S
                        ),
                        in0=pe[:].rearrange("p (b s) -> p b s", s=S),
                        in1=hd_sb[:, m, 4 * ne : 4 * ne + 4, None].to_broadcast(
                            (128, 4, S)
                        ),
                        op=OP.add,
                    )
                nc.scalar.activation(
                    out=energy[:, m, :], in_=energy[:, m, :], func=AF.Tanh,
                    bias=ba_sb[:, m : m + 1], scale=1.0,
                )

            # scores[(b s)] = energyT . v  -> [1, 1024] on one partition
            sc1p = persist.tile([1, BB * S], F32)
            for ne in range(2):
                psc = ps_att.tile([1, 512], F32, tag="att")
                for k in range(A // 128):
                    nc.tensor.matmul(
                        psc[:],
                        v_sb[:, k : k + 1],
                        energy[:, k, ne * 512 : (ne + 1) * 512],
                        start=(k == 0),
                        stop=(k == A // 128 - 1),
                    )
                nc.vector.tensor_copy(out=sc1p[:, ne * 512 : (ne + 1) * 512], in_=psc[:])

            # reshape to [8, 128] (one partition per batch row) via sbuf->sbuf DMA
            att = persist.tile([128, S], F32)     # rows 0:8 live, rest zero (for PE transpose)
            nc.vector.memset(att[:, :], 0.0)
            mask_sb = work.tile([BB, S], F32, tag="m")
            nc.sync.dma_start(out=mask_sb[:], in_=mask_d[:, :])
            # partition-scatter [1, 1024] -> [8, 128] must round-trip through DRAM
            scd = dram.tile([BB, S], F32)
            nc.sync.dma_start(
                out=scd[:].rearrange("b s -> (b s)").unsqueeze(0), in_=sc1p[:]
            )
            nc.sync.dma_start(out=att[0:BB, :], in_=scd[:])
            nc.vector.tensor_tensor(out=att[0:BB, :], in0=att[0:BB, :], in1=mask_sb[:], op=OP.add)

            # softmax over s on rows 0:8
            mx = work.tile([BB, 1], F32, tag="s1")
            nc.vector.tensor_reduce(out=mx[:], in_=att[0:BB, :], axis=AX.X, op=OP.max)
            nc.vector.tensor_tensor(
                out=att[0:BB, :], in0=att[0:BB, :],
                in1=mx[:].to_broadcast((BB, S)), op=OP.subtract,
            )
            ssum = work.tile([BB, 1], F32, tag="s2")
            nc.scalar.activation(
                out=att[0:BB, :], in_=att[0:BB, :], func=AF.Exp, accum_out=ssum[:]
            )
            rsum = work.tile([BB, 1], F32, tag="s3")
            nc.vector.reciprocal(out=rsum[:], in_=ssum[:])
            nc.vector.tensor_tensor(
                out=att[0:BB, :], in0=att[0:BB, :],
                in1=rsum[:].to_broadcast((BB, S)), op=OP.mult,
            )
            nc.sync.dma_start(out=attn_d[:, :], in_=att[0:BB, :])

            # attnT [s=128, b=8] via PE transpose
            attT = persist.tile([128, BB], F32)
            pat = ps_misc.tile([128, 128], F32, tag="mt")
            nc.tensor.transpose(pat[:], att[:], ident[:])
            nc.vector.tensor_copy(out=attT[:], in_=pat[:, 0:BB])

            # context^T [e=512, b=8]: per (e-chunk, b) matmul over s
            ctxT = persist.tile([128, Eh // 128, BB], F32)
            for o in range(Eh // 128):
                pc = ps_misc.tile([128, BB], F32, tag="mt")
                for b in range(BB):
                    nc.tensor.matmul(
                        pc[:, b : b + 1],
                        enc_sbe[:, b, o * 128 : (o + 1) * 128],
                        attT[:, b : b + 1],
                        start=True,
                        stop=True,
                    )
                nc.vector.tensor_copy(out=ctxT[:, o, :], in_=pc[:])

            # ---------------- allgather context across cores ----------------
            cc_in = dram.tile([Eh // 128, BB, 128], F32)
            nc.sync.dma_start(out=cc_in[:].rearrange("o b p -> p o b"), in_=ctxT[:])
            cc_out = dram.tile([NC * (Eh // 128), BB, 128], F32)
            nc.gpsimd.collective_compute(
                "AllGather",
                OP.bypass,
                replica_groups=[list(range(NC))],
                ins=[cc_in[:].opt()],
                outs=[cc_out[:].opt()],
            )
            ctxA = persist.tile([128, Eh // 128, B], F32)   # all-batch ctx^T
            for c in range(NC):
                for o in range(Eh // 128):
                    nc.sync.dma_start(
                        out=ctxA[:, o, c * BB : (c + 1) * BB],
                        in_=cc_out[c * (Eh // 128) + o].rearrange("b p -> p b"),
                    )

            # ---------------- GRU (replicated over full batch) --------------
            xhT = persist.tile([128, KALL, B], F32)   # [x=emb,ctx ; h]
            nc.vector.tensor_copy(out=xhT[:, 0 : E // 128, :], in_=embT[:])
            nc.vector.tensor_copy(out=xhT[:, E // 128 : KX, :], in_=ctxA[:])
            nc.sync.dma_start(
                out=xhT[:, KX:KALL, :], in_=hidT_d.rearrange("(o p) b -> p o b", p=128)
            )

            brz_sb = persist.tile([B, 2 * H], F32)
            nc.sync.dma_start(out=brz_sb[:], in_=brz_d[:, :])
            bnx_sb = work.tile([B, H], F32, tag="bx")
            nc.sync.dma_start(out=bnx_sb[:], in_=bnx_d[:, :])
            bnh_sb = work.tile([B, H], F32, tag="bh")
            nc.sync.dma_start(out=bnh_sb[:], in_=bnh_d[:, :])

            # rz = sigmoid([x;h] @ W_rz^T + b_rz)   [64, 1024]
            rz = persist.tile([B, 2 * H], F32)
            for n2 in range(2):
                prz = ps_att.tile([B, 512], F32, tag="att")
                for k in range(KALL):
                    wt = wg.tile([128, 512], F32, tag="wgk")
                    nc.sync.dma_start(
                        out=wt[:], in_=wrz_d[k * 128 : (k + 1) * 128, n2 * 512 : (n2 + 1) * 512]
                    )
                    nc.tensor.matmul(
                        prz[:], xhT[:, k, :], wt[:], start=(k == 0), stop=(k == KALL - 1)
                    )
                nc.vector.tensor_tensor(
                    out=rz[:, n2 * 512 : (n2 + 1) * 512], in0=prz[:],
                    in1=brz_sb[:, n2 * 512 : (n2 + 1) * 512], op=OP.add,
                )
            nc.scalar.activation(out=rz[:], in_=rz[:], func=AF.Sigmoid)

            # xn = x @ W_ih_n^T + b_nx ; hn = h @ W_hh_n^T + b_nh
            pxn = ps_att.tile([B, 512], F32, tag="att")
            for k in range(KX):
                wt = wg.tile([128, 512], F32, tag="wgk")
                nc.sync.dma_start(out=wt[:], in_=wnx_d[k * 128 : (k + 1) * 128, :])
                nc.tensor.matmul(pxn[:], xhT[:, k, :], wt[:], start=(k == 0), stop=(k == KX - 1))
            xn = work.tile([B, H], F32, tag="xn")
            nc.vector.tensor_tensor(out=xn[:], in0=pxn[:], in1=bnx_sb[:], op=OP.add)

            phn = ps_att.tile([B, 512], F32, tag="att")
            for k in range(KH):
                wt = wg.tile([128, 512], F32, tag="wgk")
                nc.sync.dma_start(out=wt[:], in_=wnh_d[k * 128 : (k + 1) * 128, :])
                nc.tensor.matmul(
                    phn[:], xhT[:, KX + k, :], wt[:], start=(k == 0), stop=(k == KH - 1)
                )
            hn = work.tile([B, H], F32, tag="hn")
            nc.vector.tensor_tensor(out=hn[:], in0=phn[:], in1=bnh_sb[:], op=OP.add)

            # n = tanh(xn + r*hn);  h' = n + z*(h_prev - n)
            ngate = work.tile([B, H], F32, tag="ng")
            nc.vector.tensor_tensor(out=ngate[:], in0=rz[:, 0:H], in1=hn[:], op=OP.mult)
            nc.vector.tensor_tensor(out=ngate[:], in0=ngate[:], in1=xn[:], op=OP.add)
            nc.scalar.activation(out=ngate[:], in_=ngate[:], func=AF.Tanh)

            hprev = work.tile([B, H], F32, tag="hp")
            nc.sync.dma_start(out=hprev[:], in_=hid_d[:, :])
            hnew = persist.tile([128, H], F32)    # rows 0:64 live (padded for transpose)
            nc.vector.memset(hnew[64:128, :], 0.0)
            nc.vector.tensor_tensor(out=hnew[0:B, :], in0=hprev[:], in1=ngate[:], op=OP.subtract)
            nc.vector.tensor_tensor(out=hnew[0:B, :], in0=hnew[0:B, :], in1=rz[:, H : 2 * H], op=OP.mult)
            nc.vector.tensor_tensor(out=hnew[0:B, :], in0=hnew[0:B, :], in1=ngate[:], op=OP.add)
            nc.sync.dma_start(out=hnew_d[:, :], in_=hnew[0:B, :])

            # h_new^T [512, 64] via 4 PE transposes
            hnT = persist.tile([128, KH, B], F32)
            for o in range(KH):
                pt = ps_misc.tile([128, 128], F32, tag="mt")
                nc.tensor.transpose(pt[:], hnew[:, o * 128 : (o + 1) * 128], ident[:])
                nc.vector.tensor_copy(out=hnT[:, o, :], in_=pt[:, 0:B])

            # ---------------- FC: logits = [h; ctx; emb] @ W_fc + b_fc ------
            bfc_sb = persist.tile([B, BV], F32)
            nc.sync.dma_start(out=bfc_sb[:], in_=bfc_d[:, :])
            logits_sb = persist.tile([B, BV], F32)

            # k-chunk order: emb (local early), then ctx, then h
            korder = [8, 9, 4, 5, 6, 7, 0, 1, 2, 3]

            def lhs_for(kc):
                if kc >= 8:
                    return embT[:, kc - 8, :]
                if kc >= 4:
                    return ctxA[:, kc - 4, :]
                return hnT[:, kc, :]

            NHALF = BV // 2
            for h2 in range(2):
                psl = [
                    ps_fc.tile([B, NF], F32, tag="fc", name=f"psl_{h2}_{n}")
                    for n in range(NHALF // NF)
                ]
                for ki, kc in enumerate(korder):
                    wt = wfcp.tile([128, NHALF], F32, tag="wfck")
                    nc.sync.dma_start(
                        out=wt[:],
                        in_=wfc_d[kc * 128 : (kc + 1) * 128, h2 * NHALF : (h2 + 1) * NHALF],
                    )
                    for n in range(NHALF // NF):
                        nc.tensor.matmul(
                            psl[n][:],
                            lhs_for(kc),
                            wt[:, n * NF : (n + 1) * NF],
                            start=(ki == 0),
                            stop=(ki == KALL - 1),
                        )
                for n in range(NHALF // NF):
                    off = h2 * NHALF + n * NF
                    nc.vector.tensor_tensor(
                        out=logits_sb[:, off : off + NF], in0=psl[n][:],
                        in1=bfc_sb[:, off : off + NF], op=OP.add,
                    )
            nc.sync.dma_start(out=logits_d[:, :], in_=logits_sb[:])

            if DEBUG:
                taps = {
                    "dbg_emb_rows": emb_rows,
                    "dbg_scores": sc1p,
                    "dbg_energy": energy,
                    "dbg_ctxT": ctxT,
                    "dbg_ctxA": ctxA,
                    "dbg_xhT": xhT,
                    "dbg_rz": rz,
                    "dbg_hd": hd_sb,
                    "dbg_encT": encT,
                    "dbg_embT": embT,
                }
                for name, t in taps.items():
                    o = outp(name, list(t[:].shape))
                    nc.sync.dma_start(out=o[tuple([slice(None)] * len(t[:].shape))], in_=t[:])

    _split_excess_waits(nc)
    return nc


def kernel(**inputs):
    global LAST_RESULT
    from concourse.bass_utils import run_bass_kernel_spmd

    f = lambda name: np.ascontiguousarray(np.asarray(inputs[name], dtype=np.float32))
    tokens = np.asarray(inputs["tgt_token"]).astype(np.int64)
    enc = f("encoder_outputs")
    hidden = f("hidden")
    emb = f("emb")
    W_ih, W_hh = f("W_ih"), f("W_hh")
    b_ih, b_hh = f("b_ih"), f("b_hh")
    W_fc, b_fc = f("W_fc"), f("b_fc")
    src_mask = np.asarray(inputs["src_mask"])

    hiddenT = np.ascontiguousarray(hidden.T)                      # [512, 64]
    mask_neg = np.where(src_mask, 0.0, -1e9).astype(np.float32)   # [64, 128]
    W_rzT = np.ascontiguousarray(
        np.concatenate([W_ih[: 2 * H].T, W_hh[: 2 * H].T], axis=0)
    )                                                             # [1280, 1024]
    W_nxT = np.ascontiguousarray(W_ih[2 * H :].T)                 # [768, 512]
    W_nhT = np.ascontiguousarray(W_hh[2 * H :].T)                 # [512, 512]
    b_rz_rep = np.ascontiguousarray(
        np.broadcast_to(b_ih[: 2 * H] + b_hh[: 2 * H], (B, 2 * H))
    )
    b_nx_rep = np.ascontiguousarray(np.broadcast_to(b_ih[2 * H :], (B, H)))
    b_nh_rep = np.ascontiguousarray(np.broadcast_to(b_hh[2 * H :], (B, H)))

    shared = {
        "emb": emb,
        "wa_enc": f("Wa_enc"),
        "wa_dec": f("Wa_dec"),
        "ba": f("ba"),
        "v_att": f("v_att"),
        "w_rzT": W_rzT,
        "w_nxT": W_nxT,
        "w_nhT": W_nhT,
        "b_rz_rep": b_rz_rep,
        "b_nx_rep": b_nx_rep,
        "b_nh_rep": b_nh_rep,
        "hidden": hidden,
        "hiddenT": hiddenT,
    }
    in_maps = []
    for i in range(NC):
        m = dict(shared)
        m["enc_own"] = np.ascontiguousarray(enc[i * BB : (i + 1) * BB])
        m["hT_own"] = np.ascontiguousarray(hiddenT[:, i * BB : (i + 1) * BB])
        m["mask_own"] = np.ascontiguousarray(mask_neg[i * BB : (i + 1) * BB])
        m["wfc_shard"] = np.ascontiguousarray(W_fc[:, i * BV : (i + 1) * BV])
        m["bfc_rep"] = np.ascontiguousarray(np.broadcast_to(b_fc[i * BV : (i + 1) * BV], (B, BV)))
        in_maps.append(m)

    nc = _build_program(tokens)
    import os

    tmpdir = os.environ.get("BASS_KERNEL_TRACE_DIR") or None
    res = run_bass_kernel_spmd(nc, in_maps, list(range(NC)), tmpdir=tmpdir)
    LAST_RESULT = res

    prediction = np.concatenate([res.results[i]["logits"] for i in range(NC)], axis=1)
    new_hidden = res.results[0]["new_hidden"]
    attn = np.concatenate([res.results[i]["attn_own"] for i in range(NC)], axis=0)
    return (prediction, new_hidden, attn)
